# revision 22
# baseline (speedup 1.0000x reference)
"""Trainium2 Bass kernel for the fused GNN message-passing block.

Reference computation (per batch b):
    h = silu(x @ W1 + b1) @ W2 + b2                       # [K, C]
    out[q, d, c] = sum_k mask[q,k] * ev[q,k,d] * ef[q,k,c] * h[k,c]

Sharding: data-parallel over (b, q-half) -> 8 cores, each core handles
one b (of 4) and 64 of the 128 q values.  The large per-q tensors are
staged bf16 on the host (official gate is rel_err < 2e-2; this lands
~5e-3), halving the dominant HBM stream.

Measured DMA behavior drives the structure: each dma_start costs
~0.65us of serial descriptor-gen on its queue, transfers progress in
doorbell order with a ~2us slow-start on the first transfers, and a
transfer's bandwidth is limited to the SDMA engines its source
PARTITIONS map to (engine k serves partitions {4k..4k+3, 4k+32..}).
So:
  - ALL constants ride in ONE host-packed bf16 blob DMA doorbelled
    before the ef stream on the sync (HWDGE) queue:
    [W1 | xT | b1T | W2 | evP | maskT | b2-row0].
  - the 8 ef chunk loads stream behind it on the same queue; a dummy
    Silu at the head of the scalar queue preloads the ACT table, and
    ~3.4us of bf16 warm-up matmuls flip HAM to 8/8 while they land.
  - the MLP runs entirely in bf16 (single-issue on the PE, unlike
    fp32 which double-issues); b1 rides the Silu activation's
    per-partition bias since h1T's partitions are d.
  - w[k, q, 16] = mask * evP in one DVE multiply; evP is host-packed
    with each q's d-triple at intra-group offset 4*(q%4) so the
    matmul output triples land at partitions {36s+d}, which the SBUF
    port swizzle spreads over 4 SDMA engines instead of 2 (the
    naive {32s+d} layout chokes every output DMA to ~54 GB/s).
  - main loop per 8-q chunk: DVE multiplies ef by h (bf16 2x mode,
    1.22us), one tiny matmul per q (zero-padded stationary,
    tile_position col-groups), one ACT drain per chunk into o_all;
    the last chunk is halved and drained on the idle DVE to shorten
    the tail chain.
  - 3 end-DMAs (one per d) with stride-36 partition APs write the
    whole output, spread across the sync/scalar/gpsimd queues so
    their descriptor-gens and completions overlap.

The walrus build in this container accepts at most ONE sync wait per
instruction; _split_multiwaits() hoists extra waits onto single-wait
NOPs (sequencer executes waits in queue order, so this is equivalent).
"""

import numpy as np
import ml_dtypes

import concourse.bass as bass
import concourse.mybir as mybir
import concourse.tile as tile
from concourse.bass import ds, ts
from concourse.bass_utils import run_bass_kernel_spmd

B, Q, K, D, C = 4, 128, 128, 3, 256
N_CORES = 8
QSH = Q // 2  # 64 q rows per core
QB = 8  # q values per ef chunk
NG = QSH // QB  # 8 chunks
F32 = mybir.dt.float32
BF16 = mybir.dt.bfloat16

SBLOB = 2626  # W1 512 | xT 256 | b1T 2 | W2 512 | evP 1024 | maskT 64 | b2 256 (row 0)

_NC_CACHE = {}


def _split_multiwaits(nc):
    """Legalize for the 1-sync-wait-per-instruction walrus: hoist all but
    the last wait of each instruction onto single-wait NOPs placed just
    before it on the same engine queue."""
    n = 0
    for f in nc.m.functions:
        for bb in f.blocks:
            out = []
            for inst in bb.instructions:
                si = inst.sync_info
                if si is not None and si.on_wait and len(si.on_wait) > 1:
                    waits = list(si.on_wait)
                    for w in waits[:-1]:
                        n += 1
                        nop = mybir.InstNoOp(
                            name=f"{inst.name}-wsplit{n}", ins=[], outs=[]
                        )
                        nop.engine = inst.engine
                        nop.sync_info = mybir.SyncInfo(on_wait=[w], on_update=[])
                        out.append(nop)
                    inst.sync_info = mybir.SyncInfo(
                        on_wait=[waits[-1]], on_update=list(si.on_update)
                    )
                out.append(inst)
            bb.instructions = out
    return nc


def _build_nc(split=True):
    nc = bass.Bass()

    ef_d = nc.declare_dram_parameter("efT", [K, QSH, C], BF16, isOutput=False)
    blob_d = nc.declare_dram_parameter("blob", [128, SBLOB], BF16, isOutput=False)
    out_d = nc.declare_dram_parameter("out", [4 * D, NG * 2 * C], F32, isOutput=True)

    with tile.TileContext(nc) as tc:
        with (
            tc.tile_pool(name="const", bufs=1) as cpool,
            tc.tile_pool(name="efp", bufs=1) as efpool,
            tc.tile_pool(name="outp", bufs=1) as outpool,
            tc.tile_pool(name="pprep", bufs=1, space="PSUM") as pprep,
            tc.tile_pool(name="pout", bufs=4, space="PSUM") as pout,
        ):
            # ---- sync (HWDGE) queue, strict FIFO: blobs first so they
            # land before the ef stream, then the 8 ef chunks ----
            ones_sb = cpool.tile([1, 128], BF16)
            nc.gpsimd.memset(ones_sb[:], 1.0)
            blob = cpool.tile([128, SBLOB], BF16)
            nc.sync.dma_start(blob[:], blob_d[:, :])
            ef_slots = [
                efpool.tile([K, QB, C], BF16, tag=f"ef{g}", name=f"ef{g}")
                for g in range(NG)
            ]
            for g in range(NG):
                nc.sync.dma_start(ef_slots[g][:], ef_d[:, ts(g, QB), :])

            # ---- dummy Silu on scratch: forces the ACT table load to the
            # head of the scalar queue, off the h critical path ----
            scr_out = cpool.tile([1, 128], F32)
            nc.scalar.activation(
                scr_out[:], ones_sb[:], mybir.ActivationFunctionType.Silu
            )

            # ---- PE warm-up: ~3.4us of bf16 matmuls on scratch while the
            # blob lands flips HAM to 8/8 for the MLP and main loop ----
            w_warm = cpool.tile([128, C], BF16)
            nc.gpsimd.memset(w_warm[:], 0.0)
            warm_ps = pout.tile([128, 2 * C], F32, tag="opsum", name="warm_ps")
            for _ in range(16):
                nc.tensor.matmul(
                    warm_ps[:, :C], w_warm[:, :128], w_warm[:], start=True, stop=True
                )

            # ---- MLP.  Stage 1 bf16: h1T[d, k] = (x @ W1)^T; b1 rides the
            # Silu bias (per-partition, since partitions are d here). ----
            h1T_ps = [
                pprep.tile([128, 128], F32, tag=f"prep{i}", name=f"h1T{i}")
                for i in range(2)
            ]
            for dh in range(2):
                nc.tensor.matmul(
                    h1T_ps[dh][:],
                    blob[:, ds(0 * 256 + dh * 128, 128)],
                    blob[:, ds(512 + 0 * 128, 128)],
                    start=True,
                    stop=False,
                )
                nc.tensor.matmul(
                    h1T_ps[dh][:],
                    blob[:, ds(1 * 256 + dh * 128, 128)],
                    blob[:, ds(512 + 1 * 128, 128)],
                    start=False,
                    stop=True,
                )
            h1sT_sb = cpool.tile([128, 2, 128], BF16)
            for dh in range(2):
                nc.scalar.activation(
                    h1sT_sb[:, dh],
                    h1T_ps[dh][:],
                    mybir.ActivationFunctionType.Silu,
                    bias=blob[:, ds(768 + dh, 1)],
                )
            # Stage 2 bf16: h[k, c] = h1s @ W2 + b2 (rank-1 via ones)
            h_ps = pprep.tile([128, C], F32, tag="hps", name="h_ps")
            nc.tensor.matmul(
                h_ps[:], h1sT_sb[:, 0], blob[:, ds(770, 256)], start=True, stop=False
            )
            nc.tensor.matmul(
                h_ps[:], h1sT_sb[:, 1], blob[:, ds(1026, 256)], start=False, stop=False
            )
            nc.tensor.matmul(
                h_ps[:], ones_sb[:], blob[0:1, ds(2370, 256)], start=False, stop=True
            )
            h_bf = cpool.tile([128, C], BF16)
            nc.scalar.copy(out=h_bf[:], in_=h_ps[:])

            # ---- w[k, q, 16] = mask * evP: evP is host-packed with the
            # d-triple at intra-group offset 4*(q%4), zeros elsewhere, so
            # each matmul's output triple lands at partition 36*s+d and the
            # output rows spread across 4 SDMA engines ----
            w_sb = cpool.tile([128, QSH, 16], BF16)
            nc.vector.tensor_tensor(
                w_sb[:, :, :],
                blob[:, ds(1282, 1024)].rearrange("p (q t) -> p q t", t=16),
                blob[:, ds(2306, 64)][:, :, None].to_broadcast([K, QSH, 16]),
                mybir.AluOpType.mult,
            )

            # ---- main loop over 8-q chunks; all 64 q outputs staged in
            # o_all, written out in 3 waves ----
            o_all = outpool.tile([128, NG * 2 * C], F32)
            for g in range(NG):
                ef_t = ef_slots[g]
                halves = 2 if g == NG - 1 else 1
                ps = pout.tile([128, 2 * C], F32, tag="opsum", name="ps")
                for hv in range(halves):
                    js = range(hv * QB // halves, (hv + 1) * QB // halves)
                    nc.vector.tensor_tensor(
                        ef_t[:, js.start : js.stop, :],
                        ef_t[:, js.start : js.stop, :],
                        h_bf[:, None, :].to_broadcast([K, len(js), C]),
                        mybir.AluOpType.mult,
                    )
                    for j in js:
                        f, s = j // 4, j % 4
                        q = g * QB + j
                        nc.tensor.matmul(
                            ps[ds(32 * s, 4 * s + D), ds(C * f, C)],
                            w_sb[:, q, : 4 * s + D],
                            ef_t[:, j, :],
                            start=True,
                            stop=True,
                            tile_position=(0, 32 * s),
                        )
                    drain_eng = nc.vector if g == NG - 1 else nc.scalar
                    if g == NG - 1:
                        nc.vector.tensor_copy(
                            o_all[
                                :,
                                ds(
                                    g * 2 * C + hv * (2 * C) // halves,
                                    (2 * C) // halves,
                                ),
                            ],
                            ps[:, ds(hv * (2 * C) // halves, (2 * C) // halves)],
                        )
                    else:
                        nc.scalar.copy(
                            out=o_all[
                                :,
                                ds(
                                    g * 2 * C + hv * (2 * C) // halves,
                                    (2 * C) // halves,
                                ),
                            ],
                            in_=ps[:, ds(hv * (2 * C) // halves, (2 * C) // halves)],
                        )
                if g == 6:
                    # chunks 0-6 leave now: the sync ring is idle here and
                    # these descs overlap chunk 7's compute
                    hi = 7 * 2 * C
                    for d in range(D):
                        eng = (nc.sync, nc.scalar, nc.gpsimd)[d]
                        eng.dma_start(
                            out_d[4 * d : 4 * (d + 1), :hi],
                            o_all[d : d + 109 : 36, :hi],
                        )

            # ---- final output slices (chunk 7 only; the rest left after
            # chunk 6's drain, overlapping chunk 7's compute) ----
            lo = 7 * 2 * C
            for d in range(D):
                eng = (nc.sync, nc.scalar, nc.gpsimd)[d]
                eng.dma_start(
                    out_d[4 * d : 4 * (d + 1), lo:], o_all[d : d + 109 : 36, lo:]
                )

    return _split_multiwaits(nc) if split else nc


def _get_nc():
    if "nc" not in _NC_CACHE:
        _NC_CACHE["nc"] = _build_nc()
    return _NC_CACHE["nc"]


def _in_maps(inputs):
    x = np.asarray(inputs["x"], dtype=np.float32)
    ev = np.asarray(inputs["ev"], dtype=np.float32)
    ef = np.asarray(inputs["ef"], dtype=np.float32)
    am = np.asarray(inputs["access_mask"], dtype=np.float32)
    W1 = np.asarray(inputs["W1"], dtype=np.float32)
    b1 = np.asarray(inputs["b1"], dtype=np.float32)
    W2 = np.asarray(inputs["W2"], dtype=np.float32)
    b2 = np.asarray(inputs["b2"], dtype=np.float32)
    bf = ml_dtypes.bfloat16

    blob0 = np.zeros((128, SBLOB), dtype=bf)
    for o in range(2):
        blob0[:, o * 256 : (o + 1) * 256] = W1[o * 128 : (o + 1) * 128, :].astype(bf)
        blob0[:, 770 + o * 256 : 770 + (o + 1) * 256] = W2[
            o * 128 : (o + 1) * 128, :
        ].astype(bf)
    blob0[0, 2370:2626] = b2.astype(bf)
    maps = []
    for core in range(N_CORES):
        b, qh = core // 2, core % 2
        sl = slice(qh * QSH, (qh + 1) * QSH)
        bb = blob0.copy()
        xT = x[b].T  # [C, K]
        for o in range(2):
            bb[:, 512 + o * 128 : 512 + (o + 1) * 128] = xT[
                o * 128 : (o + 1) * 128, :
            ].astype(bf)
            bb[:, 768 + o] = b1[o * 128 : (o + 1) * 128].astype(bf)
        evT = ev[b, sl].transpose(1, 2, 0)  # [K, D, QSH]
        evP = np.zeros((128, QSH, 16), dtype=bf)
        for s in range(4):
            for d in range(D):
                evP[:, s::4, 4 * s + d] = evT[:, d, s::4].astype(bf)
        bb[:, 1282:2306] = evP.reshape(128, 1024)
        bb[:, 2306:2370] = am[b, sl].T.astype(bf)
        maps.append(
            {
                "efT": np.ascontiguousarray(ef[b, sl].transpose(1, 0, 2).astype(bf)),
                "blob": bb,
            }
        )
    return maps


def _gather(results):
    out = np.empty((B, Q, D, C), dtype=np.float32)
    for core in range(N_CORES):
        b, qh = core // 2, core % 2
        # out DRAM row 4*d + s, col g*512 + f*256 + c  ->  q = g*8 + f*4 + s
        arr = results[core]["out"].reshape(D, 4, NG, 2, C)  # [d, s, g, f, c]
        out[b, qh * QSH : (qh + 1) * QSH] = (
            arr.transpose(2, 3, 1, 0, 4).reshape(QSH, D, C)
        )
    return out


def _run(inputs, trace=False, **kwargs):
    nc = _get_nc()
    res = run_bass_kernel_spmd(
        nc, _in_maps(inputs), list(range(N_CORES)), trace=trace, **kwargs
    )
    return _gather(res.results), res


def kernel(**inputs) -> np.ndarray:
    out, _ = _run(inputs, trace=False)
    return out
